# revision 34
# baseline (speedup 1.0000x reference)
"""Trainium2 Bass kernel: end-to-end model (pool -> linear -> max/argmax ->
top-k -> gather) distributed over 8 NeuronCores.

Strategy (v2): W is COLUMN-SHARDED across the 8 cores (38 of 304 padded
queries each) instead of replicated; x stays batch-sharded. Two small
collectives stitch it together:
  - AllGather of the pooled features (40KB/rank) so every core can compute
    its query-shard for ALL 64 samples, and
  - AllToAll of per-query results (68KB/rank) so every core receives its own
    8 samples x all 304 queries for the top-k + gather tail.
This cuts per-core W HBM traffic 8x (121MB -> 15.3MB) and PE moving-columns
14x vs the data-parallel baseline.

Pooling is done in ONE reduction per 32x32 cell from a host-side cell-major
uint8 packing, split across the Vector (tensor_reduce) and Activation
(accum_out) engines; the 1/(32*32*255) scale is folded into W on the host.

Self-contained: hardcodes all shapes; builds one SPMD Bass program and runs
it via run_bass_kernel_spmd on cores 0-7.
"""

import os
import sys
from contextlib import ExitStack

import numpy as np

for _p in ("/opt/trn_rl_repo", "/root/.axon_site/_ro/trn_rl_repo"):
    if os.path.isdir(_p) and _p not in sys.path:
        sys.path.append(_p)

import concourse.bass as bass
import concourse.tile as tile
from concourse import bacc, library_config, mybir
from concourse.bass_utils import run_bass_kernel_spmd

dt = mybir.dt
F32 = dt.float32
AX = mybir.AxisListType
OP = mybir.AluOpType

# ---------------- problem constants (hardcoded) ----------------
B, CHN, HIMG, WIMG = 64, 3, 640, 640
NQ, NCHAN, NCL, TOPK = 300, 84, 80, 150
KDIM, NOUT = 1200, 25200           # 3*20*20, NQ*NCHAN
NCORES = 8
BPC = B // NCORES                  # samples per core = 8
SCALE = np.float64(1.0) / (32 * 32 * 255)
NEG = -3.0e38
NIDX = 160                         # padded top-k index count (152 used)
NROUND = 19                        # 19 rounds x 8 = 152 >= 150

NQP = 304                          # padded query count (8 * 38)
QN = NQP // NCORES                 # queries per core = 38
KPAD = 1280                        # padded contraction dim (10 * 128)
KT = 10                            # k tiles of 128 rows
# chunk split of the 38 local queries (psum bank = 512 f32 >= 6*84)
CHQ = [6, 6, 6, 6, 6, 6, 2]
NCHUNKS = len(CHQ)
RG = [list(range(NCORES))]         # one replica group: all 8 cores

MM_DTYPE = F32  # kept for test.py's printout


def build_program():
    nc = bacc.Bacc("TRN2", target_bir_lowering=False, debug=False,
                   num_devices=NCORES)
    # x host-packed cell-major: partition p, free (b, tl, pix); cell
    # (b, k=tl*128+p) covers one 32x32 pool window, pix in [0,1024).
    x_d = nc.dram_tensor("x", [128, BPC * KT * 1024], dt.uint8,
                         kind="ExternalInput")
    # W shard host-packed per chunk-group: [128 krows, kt, cols] contiguous
    w6_d = nc.dram_tensor("w6", [6, 128, KT * CHQ[0] * NCHAN], F32,
                          kind="ExternalInput")
    w1_d = nc.dram_tensor("w1", [128, KT * CHQ[6] * NCHAN], F32,
                          kind="ExternalInput")
    iod_d = nc.dram_tensor("iod", [128, NCL], F32, kind="ExternalInput")
    out_d = nc.dram_tensor("out", [BPC, TOPK, 6], F32, kind="ExternalOutput")
    if os.environ.get("KERNEL_DEBUG", "0") == "1":
        dbg = {
            "dti16": nc.dram_tensor("dti16", [BPC, NIDX], dt.int16, kind="ExternalOutput"),
            "dwrap": nc.dram_tensor("dwrap", [128, NIDX // 16], dt.int16, kind="ExternalOutput"),
            "dtv": nc.dram_tensor("dtv", [BPC, NROUND * 8], F32, kind="ExternalOutput"),
            "dgout": nc.dram_tensor("dgout", [128, NIDX * 6], F32, kind="ExternalOutput"),
        }
    else:
        dbg = None

    with tile.TileContext(nc) as tc:
        with ExitStack() as ctx:
            _body(ctx, tc, x_d, w6_d, w1_d, iod_d, out_d, dbg)
    nc.finalize()
    return nc


def _body(ctx, tc, x_d, w6_d, w1_d, iod_d, out_d, dbg=None):
    nc = tc.nc

    # ---------------- persistent tiles ----------------
    P = ctx.enter_context(tc.tile_pool(name="persist", bufs=1))

    iod = P.tile([128, NCL], F32, tag="iod")
    nc.sync.dma_start(iod[:], iod_d[:])

    s_pool = P.tile([128, BPC * KT], F32, tag="s_pool")   # raw cell sums
    pg = P.tile([128, B * KT], F32, tag="pg")             # gathered pooled
    scores = P.tile([B, QN], F32, tag="scores")           # local-query scores
    a2a_sb = P.tile([B, QN * 6 + QN], F32, tag="a2a_sb")  # interleaved + scores
    eq = P.tile([B, CHQ[0] * NCL], F32, tag="eq")
    am = P.tile([B, CHQ[0] * NCL], F32, tag="am")
    argt = P.tile([B, CHQ[0]], F32, tag="argt")
    acts = P.tile([128, 1024], F32, tag="acts")           # ACT accum dump

    feat = P.tile([128, NQP * 6], F32, tag="feat")        # gather source
    swk = P.tile([BPC, NQP], F32, tag="swk")              # topk scratch
    tv = P.tile([BPC, NROUND * 8], F32, tag="tv")
    ti = P.tile([BPC, NROUND * 8], dt.uint32, tag="ti")
    ti16 = P.tile([BPC, NIDX], dt.int16, tag="ti16")
    GRAN = [(0, 48), (48, 96), (96, 160)]   # gather granules (16-aligned)
    wraps = [P.tile([128, (i1 - i0) // 16], dt.int16, tag=f"wrap{h}", name=f"wrap{h}")
             for h, (i0, i1) in enumerate(GRAN)]
    gout = P.tile([128, NIDX * 6], F32, tag="gout")

    # DRAM bounce buffers for the collectives
    DP = ctx.enter_context(tc.tile_pool(name="dram", bufs=1, space="DRAM"))
    ag_in = DP.tile([128, BPC * KT], F32, tag="ag_in")
    ag_out = DP.tile([NCORES, 128, BPC * KT], F32, tag="ag_out")
    a2a_in = DP.tile([B, QN * 7], F32, tag="a2a_in")
    a2a_out = DP.tile([NCORES, BPC, QN * 7], F32, tag="a2a_out")
    tsc = DP.tile([BPC, NIDX], dt.int16, tag="tsc")

    # load the gpsimd gather library up front (its drain overlaps pooling)
    nc.gpsimd.load_library(library_config.ap_gather)
    nc.gpsimd.memset(ti16[:, :], 0)
    nc.gpsimd.memset(feat[:, :], 0)

    # ---------------- phase 1: pooling (x -> s_pool [128, 80]) -------------
    # One 1024-wide sum per 32x32 cell. Per sample: DVE reduces the first
    # ndv tiles, ACT accumulates the rest -- the two engines run
    # concurrently. ACT is slightly slower per tile and also pays the
    # act-table load, so the last samples shift one tile to DVE.
    with tc.tile_pool(name="xp", bufs=8) as XP:
        for b in range(BPC):
            ndv = 6 if b < 6 else 5
            xh0 = XP.tile([128, 6 * 1024], dt.uint8, tag="xh0", name="xh0")
            xh1 = XP.tile([128, 5 * 1024], dt.uint8, tag="xh1", name="xh1")
            nc.sync.dma_start(
                xh0[:, : ndv * 1024], x_d[:, b * 10240 : b * 10240 + ndv * 1024]
            )
            nc.scalar.dma_start(
                xh1[:, : (KT - ndv) * 1024],
                x_d[:, b * 10240 + ndv * 1024 : (b + 1) * 10240],
            )
            with nc.allow_low_precision(reason="f32 sums of uint8 are exact"):
                nc.vector.tensor_reduce(
                    s_pool[:, b * KT : b * KT + ndv],
                    xh0[:, : ndv * 1024].rearrange("p (t x) -> p t x", x=1024),
                    axis=AX.X, op=OP.add,
                )
                for tl in range(ndv, KT):
                    nc.scalar.activation(
                        acts[:],
                        xh1[:, (tl - ndv) * 1024 : (tl - ndv + 1) * 1024],
                        mybir.ActivationFunctionType.Copy,
                        accum_out=s_pool[:, b * KT + tl : b * KT + tl + 1],
                    )

    # ---------------- W prefetch (issued before the AllGather bounce DMA so
    # the in-order DMA queues stream W during pooling, not after it) --------
    WP = ctx.enter_context(tc.tile_pool(name="wp", bufs=4))
    wgate = P.tile([1, 8], F32, tag="wgate")
    wts = []
    for g in range(NCHUNKS):
        cols = CHQ[g] * NCHAN
        wt = WP.tile([128, KT * cols], F32, tag="wt", name=f"wt{g}")
        if g == 3:
            nc.scalar.dma_start(wgate[:], ag_out[0, 0:1, 0:8])
        if g < 6:
            nc.scalar.dma_start(wt[:], w6_d[g])
        else:
            nc.scalar.dma_start(wt[:], w1_d[:])
        wts.append(wt)

    # ---------------- phase 2: AllGather pooled features -------------------
    nc.sync.dma_start(ag_in[:], s_pool[:])
    nc.gpsimd.collective_compute(
        "AllGather", OP.bypass, replica_groups=RG,
        ins=[ag_in.opt()], outs=[ag_out.opt()],
    )
    nc.sync.dma_start(
        pg[:].rearrange("p (c t) -> p c t", c=NCORES),
        ag_out[:].rearrange("c p t -> p c t"),
    )
    # lhsT tiles: pt_all[p, (t, s=(c,b))] = pooled(sample 8c+b, krow t*128+p)
    pt_all = P.tile([128, KT * B], F32, tag="pt_all")
    nc.vector.tensor_copy(
        pt_all[:].rearrange("p (t c b) -> p t c b", t=KT, c=NCORES),
        pg[:].rearrange("p (c b t) -> p t c b", c=NCORES, b=BPC),
    )

    # ---------------- phase 3: sharded matmul + per-chunk postproc ---------
    a2v = a2a_sb[:, : QN * 6].rearrange("b (q c) -> b q c", c=6)
    with tc.tile_pool(name="yps", bufs=6, space="PSUM") as YPS:
        q0 = 0
        for g in range(NCHUNKS):
            nq = CHQ[g]
            cols = nq * NCHAN
            psy = YPS.tile([B, cols], F32, tag="psy", name="psy")
            for k in range(KT):
                nc.tensor.matmul(
                    psy[:], pt_all[:, k * B : (k + 1) * B],
                    wts[g][:, k * cols : (k + 1) * cols],
                    start=(k == 0), stop=(k == KT - 1),
                )
            psv = psy[:].rearrange("b (q c) -> b q c", c=NCHAN)
            # boxes straight into the interleaved AllToAll layout
            nc.vector.tensor_copy(a2v[:, q0 : q0 + nq, 0:4], psv[:, :, 0:4])
            # per-query max score
            nc.vector.tensor_reduce(
                scores[:, q0 : q0 + nq], psv[:, :, 4:NCHAN], axis=AX.X, op=OP.max
            )
            # argmax over classes: first-index ties via iod = 79 - class_id
            eqv = eq[:, : nq * NCL].rearrange("b (q c) -> b q c", c=NCL)
            nc.vector.tensor_tensor(
                eqv, psv[:, :, 4:NCHAN],
                scores[:, q0 : q0 + nq].unsqueeze(-1).broadcast_to((B, nq, NCL)),
                op=OP.is_ge,
            )
            amv = am[:, : nq * NCL].rearrange("b (q c) -> b q c", c=NCL)
            nc.vector.tensor_tensor(
                amv, eqv,
                iod[:B, :].unsqueeze(1).broadcast_to((B, nq, NCL)),
                op=OP.mult,
            )
            nc.vector.tensor_reduce(argt[:, :nq], amv, axis=AX.X, op=OP.max)
            nc.vector.tensor_scalar(
                a2v[:, q0 : q0 + nq, 5], argt[:, :nq], -1.0, float(NCL - 1),
                op0=OP.mult, op1=OP.add,
            )
            q0 += nq

    # ---------------- phase 4: AllToAll per-query results ------------------
    nc.vector.tensor_copy(a2v[:, :, 4], scores[:])
    nc.vector.tensor_copy(a2a_sb[:, QN * 6 :], scores[:])
    nc.sync.dma_start(a2a_in[:], a2a_sb[:])
    nc.gpsimd.collective_compute(
        "AllToAll", OP.bypass, replica_groups=RG,
        ins=[a2a_in.opt()], outs=[a2a_out.opt()],
    )

    # ---------------- phase 5: top-150 tail --------------------------------
    # feat[16b] = sample b's [304, 6] rows (concat of the 8 cores' blocks)
    nc.sync.dma_start(
        feat[:].rearrange("(b s) (c x) -> b s c x", b=BPC, c=NCORES)[:, 0],
        a2a_out[:, :, : QN * 6].rearrange("c b x -> b c x"),
    )
    # swk[b, c*38+q] = score of global query c*38+q for sample b
    nc.scalar.dma_start(
        swk[:].rearrange("b (c q) -> b c q", c=NCORES),
        a2a_out[:, :, QN * 6 :].rearrange("c b q -> b c q"),
    )
    nc.vector.memset(swk[:, NQ:NQP], NEG)  # padded queries never win

    # two-half tail: indices from rounds 0..9 are wrapped + gathered while
    # rounds 10..18 still run on DVE
    def wrap_and_gather(h):
        i0, i1 = GRAN[h]
        ic = min(i1, NROUND * 8)
        nc.vector.tensor_copy(ti16[:, i0:ic], ti[:, i0:ic])
        nc.scalar.dma_start(tsc[:, i0:i1], ti16[:, i0:i1])
        for b in range(BPC):
            eng = nc.sync if b % 2 == 0 else nc.scalar
            eng.dma_start(
                wraps[h][16 * b : 16 * b + 16, :],
                tsc[b, i0:i1].rearrange("(f p) -> p f", p=16),
            )
        nc.gpsimd.ap_gather(
            gout[:].rearrange("p (i c) -> p i c", c=6)[:, i0:i1],
            feat[:].rearrange("p (q c) -> p q c", c=6),
            wraps[h][:],
            channels=128,
            num_elems=NQP,
            d=6,
            num_idxs=i1 - i0,
        )

    for r in range(NROUND):
        nc.vector.max(tv[:, 8 * r : 8 * r + 8], swk[:, :])
        nc.vector.max_index(ti[:, 8 * r : 8 * r + 8], tv[:, 8 * r : 8 * r + 8], swk[:, :])
        if r < NROUND - 1:
            nc.vector.match_replace(
                swk[:, :], tv[:, 8 * r : 8 * r + 8], swk[:, :], NEG
            )
        if r == 5:
            wrap_and_gather(0)
        elif r == 11:
            wrap_and_gather(1)
    wrap_and_gather(2)

    nc.scalar.dma_start(
        out_d[:].rearrange("b k c -> b (k c)"),
        gout[:].rearrange("(b s) x -> b s x", b=BPC)[:, 0, : TOPK * 6],
    )
    if dbg is not None:
        nc.sync.dma_start(dbg["dti16"][:], ti16[:])
        nc.sync.dma_start(dbg["dwrap"][:, 0:5], wraps[0][:])
        nc.sync.dma_start(dbg["dwrap"][:, 5:10], wraps[1][:])
        nc.sync.dma_start(dbg["dtv"][:], tv[:])
        nc.sync.dma_start(dbg["dgout"][:], gout[:])


def _make_consts():
    iod = np.broadcast_to(
        (np.float32(NCL - 1) - np.arange(NCL, dtype=np.float32))[None, :], (128, NCL)
    ).copy()
    return iod


_NC_CACHE = {}


def _get_nc():
    if "nc" not in _NC_CACHE:
        _NC_CACHE["nc"] = build_program()
    return _NC_CACHE["nc"]


def pack_x(xs: np.ndarray) -> np.ndarray:
    """[BPC, 3, 640, 640] int32 -> [128, BPC*10*1024] uint8 cell-major.

    Cell k = c_rgb*400 + i*20 + j (matching W's row layout after the
    BGR->RGB flip); cell (b, k) sits at partition k%128, free offset
    b*10240 + (k//128)*1024; cells 1200..1279 are zero padding.
    """
    xs8 = xs.astype(np.uint8).reshape(BPC, CHN, 20, 32, 20, 32)
    xs8 = xs8[:, ::-1]  # BGR -> RGB
    cells = xs8.transpose(0, 1, 2, 4, 3, 5).reshape(BPC, KDIM, 1024)
    full = np.zeros((BPC, KPAD, 1024), np.uint8)
    full[:, :KDIM] = cells
    # [b, tl, p, pix] -> [p, b, tl, pix]
    return np.ascontiguousarray(
        full.reshape(BPC, KT, 128, 1024).transpose(2, 0, 1, 3)
    ).reshape(128, BPC * KT * 1024)


def pack_w(W: np.ndarray) -> tuple[np.ndarray, np.ndarray]:
    """[1200, 25200] -> per-core chunk-group tiles (scale folded in).

    Returns (w6 [8, 6, 128, 5040], w1 [8, 128, 1680]): core c, group g holds
    [128 krows, kt, cols] for its query columns, kpad rows 1200..1279 zero.
    """
    Wp = np.zeros((KPAD, NQP * NCHAN), np.float32)
    Wp[:KDIM, : NQ * NCHAN] = (W.astype(np.float64) * SCALE).astype(np.float32)
    w6 = np.zeros((NCORES, 6, 128, KT * CHQ[0] * NCHAN), np.float32)
    w1 = np.zeros((NCORES, 128, KT * CHQ[6] * NCHAN), np.float32)
    for c in range(NCORES):
        s = Wp[:, c * QN * NCHAN : (c + 1) * QN * NCHAN]
        q0 = 0
        for g in range(NCHUNKS):
            cols = CHQ[g] * NCHAN
            blk = s[:, q0 : q0 + cols].reshape(KT, 128, cols).transpose(1, 0, 2)
            if g < 6:
                w6[c, g] = blk.reshape(128, KT * cols)
            else:
                w1[c] = blk.reshape(128, KT * cols)
            q0 += cols
    return w6, w1


def make_in_maps(x: np.ndarray, W: np.ndarray) -> list[dict]:
    iod = _make_consts()
    w6, w1 = pack_w(W)
    in_maps = []
    for c in range(NCORES):
        in_maps.append(
            {
                "x": pack_x(x[c * BPC : (c + 1) * BPC]),
                "w6": w6[c],
                "w1": w1[c],
                "iod": iod,
            }
        )
    return in_maps


def kernel(x: np.ndarray, W: np.ndarray) -> np.ndarray:
    x = np.ascontiguousarray(np.asarray(x), dtype=np.int32)
    W = np.ascontiguousarray(np.asarray(W), dtype=np.float32)
    assert x.shape == (B, CHN, HIMG, WIMG) and W.shape == (KDIM, NOUT)

    nc = _get_nc()
    in_maps = make_in_maps(x, W)
    res = run_bass_kernel_spmd(nc, in_maps, core_ids=list(range(NCORES)))
    out = np.concatenate([res.results[c]["out"] for c in range(NCORES)], axis=0)
    return out.astype(np.float32)


if __name__ == "__main__":
    xs = np.random.randint(0, 256, (B, CHN, HIMG, WIMG)).astype(np.int32)
    Ws = (np.random.randn(KDIM, NOUT) * 0.02).astype(np.float32)
    o = kernel(xs, Ws)
    print("kernel output:", o.shape, o.dtype)
